# revision 2
# baseline (speedup 1.0000x reference)
"""DGCNN forward pass on Trainium2 — Bass/Tile kernel, 8-core data parallel.

v2 redesign vs baseline: no DRAM round-trip for kNN indices.

Per graph (n=2048 points, K=20), per layer:
    t_ij = x_i . x_j - |x_j|^2 / 2          (rank-equivalent kNN score)
computed as ONE matmul pass with augmented stationary [x; ones] against
moving [x; -|x|^2/2] (layers 1-3; layer 4 uses two passes since C=128).

Selection per 128-point i-tile: 16 seg-max8 + merge -> top-24 values,
3 max_index scans -> global column indices, written INTERLEAVED into a
[128, 32ranks x 4tiles] u16 tile. Every 4 tiles one XBAR DMA transpose
(16-bit) flips it into i_allT [32 ranks, 512 points] — indices land with
rank on the partition axis, point on the free axis.

Gather trick: ap_gather consumes indices "wrapped in 16 partitions per
core": for output n, the index lives at partition n mod 16, free n div 16.
Feeding 16 rank-rows of i_allT directly as idxs gives
    out[ch, 16*i + k] = c[ch, idx[i, k]]   for ranks k=0..15
in one instruction — no rewrap needed. Ranks 16..19 come from a second
gather over i_allT rows 16..31 (rows 20..31 are zeroed; their slots are
discarded by reducing only [..., 0:4]).

Max-aggregation m = max_k c[idx] runs on GPSIMD (tensor_reduce over the
16-wide innermost axis), selection on Vector, PSUM->SBUF copies on Scalar,
matmuls on Tensor — graphs emitted back-to-back so the scheduler overlaps
graph A's aggregation with graph B's selection.

Everything stays fp32: empirically bf16/fp16 scores or aggregation push
final rel err to 1e-2..5e-2 against the 2e-2 gate.
"""

import sys
from contextlib import ExitStack

import numpy as np

sys.path.insert(0, "/opt/trn_rl_repo")

import concourse.bass as bass
from concourse import bacc
import concourse.mybir as mybir
import concourse.tile as tile

f32 = mybir.dt.float32
u16 = mybir.dt.uint16
i16 = mybir.dt.int16

NPG = 2048          # points per graph
KNN = 20            # neighbors
G = 2               # graphs per core
NCORES = 8
NT = NPG // 128     # 16 i-tiles per graph
NB = NPG // 512     # 4 moving-dim blocks per 2048
NEG = -1.0e30
SLOPE = 0.02
LATENT = 1024
AluOp = mybir.AluOpType
Act = mybir.ActivationFunctionType
AxX = mybir.AxisListType.X

LAYERS = [(3, 64), (64, 64), (64, 128), (128, 256)]
# bias_all column per (layer, chunk)
BIAS_COL = {(0, 0): 0, (1, 0): 1, (2, 0): 2, (3, 0): 3, (3, 1): 4}
# g_all (kc, row-offset) per (layer, chunk): feature f -> kc = f//128
GALL_SLOT = {(0, 0): (0, 0), (1, 0): (0, 64), (2, 0): (1, 0),
             (3, 0): (2, 0), (3, 1): (3, 0)}

AGG_BLK = 128       # points per gather block
DEBUG_DUMPS = False
NAB = NPG // AGG_BLK


def emit_prep(nc, pools, consts, g, lidx, x_in, rhsx):
    """Build rhs [x; -|x|^2/2] for the t-matmul. For L4 (C=128) rhsx row 0
    holds -|x|^2/2 and x_in itself is the x part."""
    C, O = LAYERS[lidx]
    negones = consts["negones"]
    rowp = pools["rowps"]
    sc2 = pools["sc2"]
    sqtmp = None
    if C < 128:
        nc.scalar.activation(out=rhsx[0:C, :], in_=x_in[0:C, :], func=Act.Copy)
        if C % 32 == 0:
            sqrow = rhsx[C:C + 1, :]
        else:
            # compute engines cannot write at partition base C; stage the
            # row at partition 0 and DMA it into place
            sqtmp = sc2.tile([1, NPG], f32, tag="ts", name="sqtmp")
            sqrow = sqtmp[:, :]
    else:
        sqrow = rhsx[0:1, :]
    for nb in range(NB):
        jsl = slice(512 * nb, 512 * (nb + 1))
        sqb = sc2.tile([128, 512], f32, tag="sqb", bufs=1)
        nc.scalar.activation(out=sqb[0:C, :], in_=x_in[0:C, jsl],
                             func=Act.Square)
        rps = rowp.tile([1, 512], f32, tag="row")
        nc.tensor.matmul(rps, negones[0:C, 0:1], sqb[0:C, :],
                         start=True, stop=True)
        nc.scalar.activation(out=sqrow[:, jsl], in_=rps, func=Act.Copy,
                             scale=0.5)
    if sqtmp is not None:
        nc.sync.dma_start(out=rhsx[C:C + 1, :], in_=sqtmp)


def emit_proj(nc, pools, consts, g, lidx, oc, x_in, aT, cT):
    """Chunk oc of aT = (x@Wd + b)^T and cT = (x@Wj)^T, rows 0:ow."""
    C, O = LAYERS[lidx]
    projp = pools["projps"]
    bias = consts["bias"]
    bcol = BIAS_COL[(lidx, oc)]
    if lidx <= 1:
        # merged [Wd | Wj] single pass, O=64
        wp = consts["wp"][lidx]
        for nb in range(NB):
            jsl = slice(512 * nb, 512 * (nb + 1))
            pp = projp.tile([128, 512], f32, tag="proj")
            nc.tensor.matmul(pp, wp, x_in[0:C, jsl], start=True, stop=True)
            nc.scalar.activation(out=aT[0:64, jsl], in_=pp[0:64, :],
                                 func=Act.Identity,
                                 bias=bias[0:64, bcol:bcol + 1])
            nc.scalar.activation(out=cT[0:64, jsl], in_=pp[64:128, :],
                                 func=Act.Copy)
    else:
        wd, wj = consts["wd"][lidx], consts["wj"][lidx]
        ow = min(128, O - 128 * oc)
        osl = slice(128 * oc, 128 * oc + ow)
        for nb in range(NB):
            jsl = slice(512 * nb, 512 * (nb + 1))
            pp = projp.tile([128, 512], f32, tag="proj")
            nc.tensor.matmul(pp[0:ow, :], wd[:, osl], x_in[0:C, jsl],
                             start=True, stop=True)
            nc.scalar.activation(out=aT[0:ow, jsl], in_=pp[0:ow, :],
                                 func=Act.Identity,
                                 bias=bias[0:ow, bcol:bcol + 1])
        for nb in range(NB):
            jsl = slice(512 * nb, 512 * (nb + 1))
            pp = projp.tile([128, 512], f32, tag="proj")
            nc.tensor.matmul(pp[0:ow, :], wj[:, osl], x_in[0:C, jsl],
                             start=True, stop=True)
            nc.scalar.activation(out=cT[0:ow, jsl], in_=pp[0:ow, :],
                                 func=Act.Copy)


def emit_knn(nc, pools, consts, g, lidx, x_in, rhsx, i_allT,
             dbg=None):
    """t-matmul + top-24 selection per i-tile; indices into i_allT
    [32 ranks, 16 tiles, 128] u16 via interleaved XBAR DMA transposes."""
    C, O = LAYERS[lidx]
    bigps = pools["bigps"]
    sc2 = pools["sc2"]
    selp = pools["sel"]
    ones = consts["ones"]
    i4 = None
    for T in range(NT):
        b = T % 4
        if b == 0:
            i4 = selp.tile([128, 4, 32], u16, tag="i4")
            nc.gpsimd.memset(i4, 0)
        xsl = slice(128 * T, 128 * (T + 1))
        t_ps = bigps.tile([128, NPG], f32, tag="t")
        if C < 128:
            for nb in range(NB):
                jsl = slice(512 * nb, 512 * (nb + 1))
                nc.tensor.matmul(t_ps[:, jsl], x_in[0:C + 1, xsl],
                                 rhsx[0:C + 1, jsl], start=True, stop=True)
        else:
            for nb in range(NB):
                jsl = slice(512 * nb, 512 * (nb + 1))
                nc.tensor.matmul(t_ps[:, jsl], x_in[:, xsl],
                                 x_in[:, jsl], start=True, stop=False)
            for nb in range(NB):
                jsl = slice(512 * nb, 512 * (nb + 1))
                nc.tensor.matmul(t_ps[:, jsl], ones[0:1, 0:128],
                                 rhsx[0:1, jsl], start=False, stop=True)
        ts = sc2.tile([128, NPG], f32, tag="ts")
        nc.scalar.activation(out=ts, in_=t_ps, func=Act.Copy)
        if dbg is not None and T == 0:
            nc.sync.dma_start(out=dbg["ts"][:, :], in_=ts)
        cand = selp.tile([128, 128], f32, tag="cand")
        for s in range(16):
            nc.vector.max(out=cand[:, 8 * s:8 * s + 8],
                          in_=ts[:, 128 * s:128 * (s + 1)])
        v = selp.tile([128, 24], f32, tag="v")
        nc.vector.max(out=v[:, 0:8], in_=cand)
        nc.vector.match_replace(out=cand, in_to_replace=v[:, 0:8],
                                in_values=cand, imm_value=NEG)
        nc.vector.max(out=v[:, 8:16], in_=cand)
        nc.vector.match_replace(out=cand, in_to_replace=v[:, 8:16],
                                in_values=cand, imm_value=NEG)
        nc.vector.max(out=v[:, 16:24], in_=cand)
        nc.vector.max_index(out=i4[:, b, 0:8], in_max=v[:, 0:8], in_values=ts)
        nc.vector.max_index(out=i4[:, b, 8:16], in_max=v[:, 8:16],
                            in_values=ts)
        nc.vector.max_index(out=i4[:, b, 16:24], in_max=v[:, 16:24],
                            in_values=ts)
        if b == 3:
            g4 = T // 4
            if dbg is not None and g4 == 0:
                nc.sync.dma_start(out=dbg["i4"][:, :, :], in_=i4[:, :, :])
            # HW XBAR honors only the plain 2D [128,128] transpose: row of
            # iTT = source column = 32*b2 + rank. Reassemble per tile.
            iTT = sc2.tile([128, 128], u16, tag="iTT", bufs=1)
            nc.sync.dma_start(out=iTT[:, :], in_=i4[:, :, :], transpose=True)
            for b2 in range(4):
                nc.sync.dma_start(out=i_allT[:, 4 * g4 + b2, :],
                                  in_=iTT[32 * b2:32 * b2 + 32, :])


def emit_repl(nc, pools, i_allT):
    """Replicate i_allT rank rows across the 8 GPSIMD-core partition groups."""
    sc2 = pools["sc2"]
    R1 = sc2.tile([128, NPG], u16, tag="R1", bufs=1)
    R2 = sc2.tile([128, NPG], u16, tag="R2", bufs=1)
    iflat = i_allT.rearrange("r t j -> r (t j)")
    for grp in range(8):
        psl = slice(16 * grp, 16 * grp + 16)
        nc.sync.dma_start(out=R1[psl, :], in_=iflat[0:16, :])
        nc.sync.dma_start(out=R2[psl, :], in_=iflat[16:32, :])
    return R1, R2


def emit_agg(nc, pools, g, lidx, oc, R1, R2, aT, cT, x_out, g_all):
    """Gather + max-aggregate chunk oc, apply leaky, write next-layer
    features / pool slots."""
    C, O = LAYERS[lidx]
    sc2 = pools["sc2"]
    selp = pools["sel"]
    ow = min(128, O - 128 * oc)
    m = pools["state"].tile([128, NPG], f32, tag=f"m_g{g}", name=f"m_g{g}")
    for blk in range(NAB):
        isl = slice(AGG_BLK * blk, AGG_BLK * (blk + 1))
        gout = sc2.tile([128, AGG_BLK, 16], f32, tag="gout")
        nc.gpsimd.ap_gather(
            out_ap=gout[0:ow], in_ap=cT[0:ow, :],
            idxs_ap=R1[0:ow, isl].bitcast(i16), channels=ow,
            num_elems=NPG, d=1, num_idxs=AGG_BLK * 16)
        nc.vector.tensor_reduce(out=m[0:ow, isl], in_=gout[0:ow],
                                axis=AxX, op=AluOp.max)
        gout2 = sc2.tile([128, AGG_BLK, 16], f32, tag="gout")
        nc.gpsimd.ap_gather(
            out_ap=gout2[0:ow], in_ap=cT[0:ow, :],
            idxs_ap=R2[0:ow, isl].bitcast(i16), channels=ow,
            num_elems=NPG, d=1, num_idxs=AGG_BLK * 16)
        m2 = sc2.tile([128, AGG_BLK], f32, tag="m2")
        nc.vector.tensor_reduce(out=m2[0:ow, :], in_=gout2[0:ow, :, 0:4],
                                axis=AxX, op=AluOp.max)
        nc.vector.tensor_tensor(out=m[0:ow, isl], in0=m[0:ow, isl],
                                in1=m2[0:ow, :], op=AluOp.max)
    # m = a + m
    nc.vector.tensor_tensor(out=m[0:ow, :], in0=m[0:ow, :],
                            in1=aT[0:ow, :], op=AluOp.add)
    kc, roff = GALL_SLOT[(lidx, oc)]
    if lidx < 3:
        # xn = leaky(m) -> next-layer features; pool from xn
        nc.vector.scalar_tensor_tensor(out=x_out[0:O, :], in0=m[0:ow, :],
                                       scalar=SLOPE, in1=m[0:ow, :],
                                       op0=AluOp.mult, op1=AluOp.max)
        if roff == 0:
            nc.vector.tensor_reduce(out=g_all[0:ow, kc:kc + 1, g],
                                    in_=x_out[0:O, :], axis=AxX,
                                    op=AluOp.max)
        else:
            ptmp = selp.tile([64, 1], f32, tag="ptmp")
            nc.vector.tensor_reduce(out=ptmp, in_=x_out[0:O, :],
                                    axis=AxX, op=AluOp.max)
            nc.sync.dma_start(out=g_all[roff:roff + ow, kc:kc + 1, g],
                              in_=ptmp)
    else:
        # last layer: pool-max commutes with monotone leaky
        ptmp = selp.tile([128, 1], f32, tag="ptmp4")
        nc.vector.tensor_reduce(out=ptmp[0:ow, :], in_=m[0:ow, :],
                                axis=AxX, op=AluOp.max)
        nc.vector.scalar_tensor_tensor(out=g_all[0:ow, kc:kc + 1, g],
                                       in0=ptmp[0:ow, :], scalar=SLOPE,
                                       in1=ptmp[0:ow, :],
                                       op0=AluOp.mult, op1=AluOp.max)


def build_nc():
    nc = bacc.Bacc()
    posT = nc.declare_dram_parameter("posT", [3, G * NPG], f32, isOutput=False)
    wp_d = [nc.declare_dram_parameter(f"wp{l}", [LAYERS[l][0], 128], f32,
                                      isOutput=False) for l in range(2)]
    wd_d = {l: nc.declare_dram_parameter(f"wd{l}", list(LAYERS[l]), f32,
                                         isOutput=False) for l in (2, 3)}
    wj_d = {l: nc.declare_dram_parameter(f"wj{l}", list(LAYERS[l]), f32,
                                         isOutput=False) for l in (2, 3)}
    bias_d = nc.declare_dram_parameter("bias_all", [128, 5], f32,
                                       isOutput=False)
    wl_d = nc.declare_dram_parameter("wl", [512, LATENT], f32, isOutput=False)
    bl_d = nc.declare_dram_parameter("bl", [1, LATENT], f32, isOutput=False)
    out_d = nc.declare_dram_parameter("out", [G, LATENT], f32, isOutput=True)
    dbg = {}
    if DEBUG_DUMPS:
        dbg["ts"] = nc.declare_dram_parameter("d_ts", [128, NPG], f32,
                                              isOutput=True)
        dbg["iT"] = nc.declare_dram_parameter("d_iT", [32, NT, 128], u16,
                                              isOutput=True)
        dbg["aT"] = nc.declare_dram_parameter("d_aT", [64, NPG], f32,
                                              isOutput=True)
        dbg["cT"] = nc.declare_dram_parameter("d_cT", [64, NPG], f32,
                                              isOutput=True)
        dbg["x1"] = nc.declare_dram_parameter("d_x1", [65, NPG], f32,
                                              isOutput=True)
        dbg["R1"] = nc.declare_dram_parameter("d_R1", [128, NPG], u16,
                                              isOutput=True)
        dbg["i4"] = nc.declare_dram_parameter("d_i4", [128, 4, 32], u16,
                                              isOutput=True)

    with tile.TileContext(nc) as tc, ExitStack() as ctx:
        const = ctx.enter_context(tc.tile_pool(name="const", bufs=1))
        state = ctx.enter_context(tc.tile_pool(name="state", bufs=1))
        selp = ctx.enter_context(tc.tile_pool(name="selp", bufs=2))
        sc2 = ctx.enter_context(tc.tile_pool(name="sc2", bufs=2))
        bigps = ctx.enter_context(tc.tile_pool(name="bigps", bufs=1,
                                               space="PSUM"))
        projps = ctx.enter_context(tc.tile_pool(name="projps", bufs=2,
                                                space="PSUM"))
        rowps = ctx.enter_context(tc.tile_pool(name="rowps", bufs=2,
                                               space="PSUM"))
        pools = {"state": state, "sel": selp, "sc2": sc2, "bigps": bigps,
                 "projps": projps, "rowps": rowps}

        ones = const.tile([1, 128], f32)
        nc.vector.memset(ones, 1.0)
        negones = const.tile([128, 1], f32)
        nc.vector.memset(negones, -1.0)
        wp = [const.tile_from(wp_d[l][:, :], name=f"wp{l}s") for l in range(2)]
        wd = {l: const.tile_from(wd_d[l][:, :], name=f"wd{l}s") for l in (2, 3)}
        wj = {l: const.tile_from(wj_d[l][:, :], name=f"wj{l}s") for l in (2, 3)}
        bias = const.tile_from(bias_d[:, :], name="bias_s")
        bls = const.tile_from(bl_d[:, :], name="bls")
        consts = {"ones": ones, "negones": negones, "wp": wp, "wd": wd,
                  "wj": wj, "bias": bias}

        g_all = const.tile([128, 4, G], f32)
        out_sb = const.tile([G, LATENT], f32)

        # per-graph ping-pong feature tiles
        xa = [state.tile([128, NPG], f32, tag=f"xa_g{g}", name=f"xa_g{g}")
              for g in range(G)]
        row1 = const.tile([1, NPG], f32)
        nc.vector.memset(row1, 1.0)
        for g in range(G):
            nc.sync.dma_start(out=xa[g][0:3, :],
                              in_=posT[:, g * NPG:(g + 1) * NPG])
            nc.sync.dma_start(out=xa[g][3:4, :], in_=row1)

        x_cur = list(xa)
        for lidx, (C, O) in enumerate(LAYERS):
            nocs = (O + 127) // 128
            for g in range(G):
                x_in = x_cur[g]
                x_out = None
                if lidx < 3:
                    x_out = state.tile([128, NPG], f32,
                                       tag=f"x{'ab'[(lidx + 1) % 2]}_g{g}",
                                       name=f"xout{lidx}_g{g}")
                x_cur[g] = x_out
                rhsx = sc2.tile([128, NPG], f32, tag="rhsx")
                aTs = [state.tile([128, NPG], f32, tag=f"aT{oc}_g{g}",
                                  name=f"aT{lidx}_{oc}_g{g}")
                       for oc in range(nocs)]
                cTs = [state.tile([128, NPG], f32, tag=f"cT{oc}_g{g}",
                                  name=f"cT{lidx}_{oc}_g{g}")
                       for oc in range(nocs)]
                i_allT = sc2.tile([32, NT, 128], u16, tag="iT")
                emit_prep(nc, pools, consts, g, lidx, x_in, rhsx)
                for oc in range(nocs):
                    emit_proj(nc, pools, consts, g, lidx, oc, x_in,
                              aTs[oc], cTs[oc])
                emit_knn(nc, pools, consts, g, lidx, x_in, rhsx, i_allT,
                         dbg if (DEBUG_DUMPS and lidx == 0 and g == 0)
                         else None)
                R1, R2 = emit_repl(nc, pools, i_allT)
                if DEBUG_DUMPS and lidx == 0 and g == 0:
                    nc.sync.dma_start(out=dbg["iT"][:, :, :], in_=i_allT)
                    nc.sync.dma_start(out=dbg["R1"][:, :], in_=R1)
                    nc.sync.dma_start(out=dbg["aT"][:, :], in_=aTs[0][0:64, :])
                    nc.sync.dma_start(out=dbg["cT"][:, :], in_=cTs[0][0:64, :])
                for oc in range(nocs):
                    emit_agg(nc, pools, g, lidx, oc, R1, R2,
                             aTs[oc], cTs[oc], x_out, g_all)
                if x_out is not None and O < 128:
                    nc.vector.memset(x_out[O:O + 1, :], 1.0)
                if DEBUG_DUMPS and lidx == 0 and g == 0:
                    nc.sync.dma_start(out=dbg["x1"][:, :], in_=x_out[0:65, :])

        # final MLP: stream wl in [128, 512] chunks
        for nb in range(LATENT // 512):
            nsl = slice(512 * nb, 512 * (nb + 1))
            fo = projps.tile([128, 512], f32, tag="proj")
            for kc in range(4):
                wch = sc2.tile([128, 512], f32, tag="wlch")
                nc.sync.dma_start(out=wch,
                                  in_=wl_d[128 * kc:128 * (kc + 1), nsl])
                nc.tensor.matmul(fo[0:G, :], g_all[:, kc, :], wch,
                                 start=(kc == 0), stop=False)
            nc.tensor.matmul(fo[0:G, :], ones[0:1, 0:G], bls[:, nsl],
                             start=False, stop=True)
            nc.scalar.activation(out=out_sb[:, nsl], in_=fo[0:G, :],
                                 func=Act.Relu)
        nc.sync.dma_start(out=out_d[:, :], in_=out_sb)

    nc.finalize()
    return nc


# ---------------------------------------------------------------------------
_NC_CACHE = {}


def _get_nc():
    if "v2" not in _NC_CACHE:
        _NC_CACHE["v2"] = build_nc()
    return _NC_CACHE["v2"]


def make_in_maps(inputs):
    pos = np.ascontiguousarray(np.asarray(inputs["pos"], dtype=np.float32))
    Ws = [np.asarray(inputs[f"W{i}"], np.float32) for i in range(1, 5)]
    bs = [np.asarray(inputs[f"b{i}"], np.float32) for i in range(1, 5)]
    wl = np.ascontiguousarray(np.asarray(inputs["Wl"], np.float32))
    bl = np.ascontiguousarray(np.asarray(inputs["bl"], np.float32)[None, :])
    base = {"wl": wl, "bl": bl}
    bias_all = np.zeros((128, 5), np.float32)
    for l, (C, O) in enumerate(LAYERS):
        W, b = Ws[l], bs[l]
        Wd = np.ascontiguousarray(W[:C] - W[C:])
        Wj = np.ascontiguousarray(W[C:])
        if l <= 1:
            base[f"wp{l}"] = np.ascontiguousarray(
                np.concatenate([Wd, Wj], axis=1))
        else:
            base[f"wd{l}"] = Wd
            base[f"wj{l}"] = Wj
        if O <= 128:
            bias_all[0:O, BIAS_COL[(l, 0)]] = b
        else:
            bias_all[:, BIAS_COL[(l, 0)]] = b[0:128]
            bias_all[:, BIAS_COL[(l, 1)]] = b[128:256]
    base["bias_all"] = bias_all
    in_maps = []
    for c in range(NCORES):
        m = dict(base)
        m["posT"] = np.ascontiguousarray(
            pos[c * G * NPG:(c + 1) * G * NPG].T)
        in_maps.append(m)
    return in_maps


def kernel(**inputs) -> np.ndarray:
    from concourse.bass_utils import run_bass_kernel_spmd
    nc = _get_nc()
    in_maps = make_in_maps(inputs)
    res = run_bass_kernel_spmd(nc, in_maps, list(range(NCORES)))
    return np.concatenate([r["out"] for r in res.results], axis=0)


if __name__ == "__main__":
    nc = build_nc()
    print("build OK")
